# revision 10
# baseline (speedup 1.0000x reference)
"""Trainium2 Bass kernel for MiniMHCLM.

Math (HC=4, C=512, K=HC*C=2048, VOCAB=32000, tokens N=B*S=4096):
  x = embed[ids]                               [N, K]
  invr = rsqrt(mean(x^2, -1) + eps)
  mix = (x @ phi) * invr                       [N, 24]
  h_pre  = sigmoid(mix[:, :4]*a_pre + b[:4]) + 0.01
  h_post = sigmoid(mix[:, 4:8]*a_post + b[4:8]) * 2
  h_res  = sinkhorn(mix[:, 8:24]*a_res + b[8:24], 8 iters)  [N,4,4]
  x_in  = sum_i h_pre[i] * x[:, i*C:(i+1)*C]
  f_out = x_in @ W_inner.T
  x_out[o] = sum_i h_res[o,i]*x[i] + h_post[o]*f_out
  logits = x_out.reshape(N, K) @ W_head.T      [N, VOCAB]

Distribution: all 8 cores run the identical coeff path over all tokens;
the head projection is column-sharded over vocab (4000 per core).

PE strategy: all heavy matmuls use the float32r dtype (fp32 stored,
PE rounds operands to 11 mantissa bits, runs at 1 cyc/row for moving
dim >= 256 - 4x faster than plain fp32, ~25x more accurate than bf16).
Per-token coefficient scaling is done ON the PE by using diag(h) as the
stationary operand: diag(h).T @ x = per-token-row scaled x, accumulated
in PSUM across hyper-channels. POST_MULT=2 is folded into W_inner.
"""

import sys

for _p in ("/opt/trn_rl_repo", "/root/.axon_site/_ro/trn_rl_repo"):
    if _p not in sys.path:
        sys.path.insert(0, _p)

import numpy as np

import concourse.bass as bass
import concourse.mybir as mybir
import concourse.tile as tile
from concourse.bass_utils import run_bass_kernel_spmd

F32 = mybir.dt.float32
F32R = mybir.dt.float32r
ALU = mybir.AluOpType
ACTF = mybir.ActivationFunctionType
AX = mybir.AxisListType

P = 128
HC, C = 4, 512
K = HC * C  # 2048
KS = K // P  # 16
M = HC * HC + 2 * HC  # 24
RMS_EPS = 1e-6
PRE_EPS = 0.01
TMAX = 8
N_CORES = 8


def legalize_multiwait(nc):
    """Split instructions carrying >1 semaphore wait.

    The walrus build in this image rejects instructions with more than
    one sem wait ("Too many sync wait commands"); Tile emits them
    freely. Move all but the last wait onto standalone InstEventSemaphore
    instructions inserted just before, on the same engine.
    """
    n_fixed = 0
    for fn in nc.m.functions:
        for blk in fn.blocks:
            new = []
            for ins in blk.instructions:
                si = ins.sync_info
                if si is not None and si.on_wait and len(si.on_wait) > 1:
                    waits = list(si.on_wait)
                    for j, w in enumerate(waits[:-1]):
                        es = mybir.InstEventSemaphore(
                            name=f"{ins.name}-w{j}",
                            ins=[],
                            outs=[],
                            sync_info=mybir.SyncInfo(on_wait=[w], on_update=[]),
                        )
                        es.engine = ins.engine
                        nc.register_instruction(es)
                        new.append(es)
                        n_fixed += 1
                    ins.sync_info = mybir.SyncInfo(
                        on_wait=[waits[-1]], on_update=list(si.on_update)
                    )
                new.append(ins)
            blk.instructions[:] = new
    return n_fixed


def build_nc(n_tok, vsh, embed_rows, group_tiles, vt_size=400):
    """Build the single-core Bass program (same on every core; only the
    wht input shard differs per core).

    The per-tile coefficient work is software-pipelined with a 1-tile
    skew: PE runs tile t's mixing/merge stage while DVE/ACT compute
    tile t+1's coefficients, keeping the PE dense (HAM stays warm).
    """
    n_tiles = n_tok // P
    assert n_tiles % group_tiles == 0
    n_groups = n_tiles // group_tiles
    assert vsh % vt_size == 0
    n_vt = vsh // vt_size

    nc = bass.Bass()

    ids_d = nc.dram_tensor("ids", [P, n_tiles], mybir.dt.int32, kind="ExternalInput")
    embed_d = nc.dram_tensor("embed", [embed_rows, K], F32R, kind="ExternalInput")
    wht_d = nc.dram_tensor("wht", [K, vsh], F32R, kind="ExternalInput")
    winner_d = nc.dram_tensor("winner", [C, C], F32R, kind="ExternalInput")
    phi_d = nc.dram_tensor("phi", [K, M], F32R, kind="ExternalInput")
    params_d = nc.dram_tensor("params", [P, 28], F32, kind="ExternalInput")
    ident_d = nc.dram_tensor("ident", [P, P], F32, kind="ExternalInput")
    out_d = nc.dram_tensor("out", [n_tok, vsh], F32, kind="ExternalOutput")

    wht_v = wht_d[:].rearrange("(ko p) v -> p ko v", p=P)  # [128, 16, vsh]

    with tile.TileContext(nc) as tc:
        with (
            tc.tile_pool(name="const", bufs=1) as cpool,
            tc.tile_pool(name="xg", bufs=2) as xg,
            tc.tile_pool(name="xg2", bufs=2) as xg2,
            tc.tile_pool(name="gsm", bufs=1) as gsm,
            tc.tile_pool(name="sm", bufs=2) as sm,
            tc.tile_pool(name="dg", bufs=1) as dgp,
            tc.tile_pool(name="xt", bufs=1) as xtp,
            tc.tile_pool(name="xin", bufs=2) as xinp,
            tc.tile_pool(name="fo", bufs=2) as fop,
            tc.tile_pool(name="xo", bufs=1) as xop,
            tc.tile_pool(name="xmt", bufs=group_tiles) as xmtp,
            tc.tile_pool(name="wp", bufs=2) as wp,
            tc.tile_pool(name="ost", bufs=2) as ostp,
            tc.tile_pool(name="ps_tp", bufs=2, space="PSUM") as ps_tp,
            tc.tile_pool(name="ps_acc", bufs=2, space="PSUM") as ps_acc,
            tc.tile_pool(name="ps_head", bufs=3, space="PSUM") as ps_head,
        ):
            # ---- constants ----
            phi_sb = cpool.tile([P, KS, M], F32R)
            nc.sync.dma_start(phi_sb[:], phi_d[:].rearrange("(ko p) m -> p ko m", p=P))
            winner_sb = cpool.tile([P, HC, C], F32R)
            nc.sync.dma_start(
                winner_sb[:], winner_d[:].rearrange("(ko p) c -> p ko c", p=P)
            )
            bvec = cpool.tile([P, 28], F32)
            nc.sync.dma_start(bvec[:], params_d[:])
            ids_sb = cpool.tile([P, n_tiles], mybir.dt.int32)
            nc.sync.dma_start(ids_sb[:], ids_d[:])
            ident_sb = cpool.tile([P, P], F32)
            nc.sync.dma_start(ident_sb[:], ident_d[:])
            ident_r = cpool.tile([P, P], F32R)
            nc.vector.tensor_copy(out=ident_r[:], in_=ident_sb[:])

            def stage1a(tt, t8, gssq4, gmix):
                """Gather + RMS partials + x^T + mix matmul -> gmix slice."""
                x_r = xg.tile([P, K], F32R, tag="x")
                nc.gpsimd.indirect_dma_start(
                    out=x_r[:],
                    out_offset=None,
                    in_=embed_d[:],
                    in_offset=bass.IndirectOffsetOnAxis(
                        ap=ids_sb[:, tt : tt + 1], axis=0
                    ),
                )
                x_f = x_r[:].bitcast(F32)

                # RMS sum-of-squares (ACT square + accum, chunked)
                ssq4 = sm.tile([P, HC], F32, tag="ssq4")
                for q in range(HC):
                    scratch = xinp.tile([P, C], F32, tag="xtmp")
                    nc.scalar.activation(
                        out=scratch[:],
                        in_=x_f[:, q * C : (q + 1) * C],
                        func=ACTF.Square,
                        accum_out=ssq4[:, q : q + 1],
                    )
                nc.vector.tensor_reduce(
                    out=gssq4[:, t8 : t8 + 1], in_=ssq4[:], axis=AX.X, op=ALU.add
                )

                # transpose x (for the mix matmul); copies on ACT
                xT = xtp.tile([P, KS, P], F32R, tag="xT")
                for kb in range(KS // 4):
                    pt = ps_tp.tile([P, 4 * P], F32R, tag="tp")
                    for j in range(4):
                        ks = kb * 4 + j
                        nc.tensor.transpose(
                            pt[:, j * P : (j + 1) * P],
                            x_r[:, ks * P : (ks + 1) * P],
                            ident_r[:],
                        )
                    nc.scalar.copy(
                        out=xT[:, 4 * kb : 4 * kb + 4, :].rearrange(
                            "p a b -> p (a b)"
                        ),
                        in_=pt[:],
                    )

                # mix = x @ phi (invr applied later, batched)
                pm = ps_acc.tile([P, C], F32, tag="acc")
                for ks in range(KS):
                    nc.tensor.matmul(
                        pm[:, :M],
                        xT[:, ks, :],
                        phi_sb[:, ks, :],
                        start=(ks == 0),
                        stop=(ks == KS - 1),
                    )
                nc.vector.tensor_copy(out=gmix[:, t8, :], in_=pm[:, :M])

            def batched_coeffs(gt, gssq4, gmix):
                """All small coefficient math for a whole group at once."""
                rms_g = gsm.tile([P, gt], F32, tag="rms_g")
                nc.scalar.activation(
                    out=rms_g[:], in_=gssq4[:], func=ACTF.Sqrt, scale=1.0 / K,
                    bias=bvec[:, 27:28],
                )
                invr_g = gsm.tile([P, gt], F32, tag="invr_g")
                nc.vector.reciprocal(out=invr_g[:], in_=rms_g[:])
                nc.vector.tensor_tensor(
                    out=gmix[:], in0=gmix[:],
                    in1=invr_g[:, :, None].to_broadcast([P, gt, M]),
                    op=ALU.mult,
                )
                lg = gsm.tile([P, gt, M], F32, tag="lg_g")
                nc.vector.tensor_scalar(
                    out=lg[:, :, 0:4], in0=gmix[:, :, 0:4],
                    scalar1=bvec[:, 24:25], scalar2=None, op0=ALU.mult,
                )
                nc.vector.tensor_scalar(
                    out=lg[:, :, 4:8], in0=gmix[:, :, 4:8],
                    scalar1=bvec[:, 25:26], scalar2=None, op0=ALU.mult,
                )
                nc.vector.tensor_scalar(
                    out=lg[:, :, 8:24], in0=gmix[:, :, 8:24],
                    scalar1=bvec[:, 26:27], scalar2=None, op0=ALU.mult,
                )
                nc.vector.tensor_tensor(
                    out=lg[:], in0=lg[:],
                    in1=bvec[:, None, 0:24].to_broadcast([P, gt, M]),
                    op=ALU.add,
                )
                sg_g = gsm.tile([P, gt, 8], F32, tag="sg_g")
                nc.scalar.activation(
                    out=sg_g[:], in_=lg[:, :, 0:8], func=ACTF.Sigmoid
                )
                hpre_g = gsm.tile([P, gt, 4], F32, tag="hpre_g")
                nc.vector.tensor_scalar(
                    out=hpre_g[:], in0=sg_g[:, :, 0:4], scalar1=PRE_EPS,
                    scalar2=None, op0=ALU.add,
                )
                mat_g = gsm.tile([P, gt, HC, HC], F32, tag="mat_g")  # [p,t,o,i]
                nc.scalar.activation(
                    out=mat_g[:].rearrange("p t o i -> p t (o i)"),
                    in_=lg[:, :, 8:24],
                    func=ACTF.Exp,
                )
                rsum = gsm.tile([P, gt, HC], F32, tag="rsum_g")
                rrec = gsm.tile([P, gt, HC], F32, tag="rrec_g")

                def row_norm():
                    nc.vector.tensor_reduce(
                        out=rsum[:], in_=mat_g[:], axis=AX.X, op=ALU.add
                    )
                    nc.vector.reciprocal(out=rrec[:], in_=rsum[:])
                    nc.vector.tensor_tensor(
                        out=mat_g[:], in0=mat_g[:],
                        in1=rrec[:, :, :, None].to_broadcast([P, gt, HC, HC]),
                        op=ALU.mult,
                    )

                def col_norm():
                    nc.vector.tensor_reduce(
                        out=rsum[:], in_=mat_g[:].rearrange("p t o i -> p t i o"),
                        axis=AX.X, op=ALU.add,
                    )
                    nc.vector.reciprocal(out=rrec[:], in_=rsum[:])
                    nc.vector.tensor_tensor(
                        out=mat_g[:], in0=mat_g[:],
                        in1=rrec[:, :, None, :].to_broadcast([P, gt, HC, HC]),
                        op=ALU.mult,
                    )

                row_norm()  # softmax denominator
                col_norm()
                for _ in range(TMAX - 1):
                    row_norm()
                    col_norm()
                return sg_g, hpre_g, mat_g

            def stage2(tt, t8, sg_g, hpre_g, mat_g):
                """Re-gather x, then diag + x_in + f_out + mixing + merge."""
                x_r = xg2.tile([P, K], F32R, tag="x2")
                nc.gpsimd.indirect_dma_start(
                    out=x_r[:],
                    out_offset=None,
                    in_=embed_d[:],
                    in_offset=bass.IndirectOffsetOnAxis(
                        ap=ids_sb[:, tt : tt + 1], axis=0
                    ),
                )
                x_f = x_r[:].bitcast(F32)

                # diag coefficient tile: [h_res(16) | h_post(4)]
                dg = dgp.tile([P, 20, P], F32R, tag="dg")
                nc.vector.tensor_tensor(
                    out=dg[:, 0:16, :],
                    in0=ident_sb[:, None, :].to_broadcast([P, 16, P]),
                    in1=mat_g[:, t8].rearrange("p o i -> p (o i)")[:, :, None]
                    .to_broadcast([P, 16, P]),
                    op=ALU.mult,
                )
                nc.vector.tensor_tensor(
                    out=dg[:, 16:20, :],
                    in0=ident_sb[:, None, :].to_broadcast([P, 4, P]),
                    in1=sg_g[:, t8, 4:8, None].to_broadcast([P, 4, P]),
                    op=ALU.mult,
                )

                # x_in = sum_i h_pre[i] * x_i
                xi = xinp.tile([P, C], F32, tag="xi")
                nc.vector.tensor_scalar(
                    out=xi[:], in0=x_f[:, 0:C], scalar1=hpre_g[:, t8, 0:1],
                    scalar2=None, op0=ALU.mult,
                )
                xtmp = xinp.tile([P, C], F32, tag="xtmp")
                for i in range(1, HC):
                    nc.vector.tensor_scalar(
                        out=xtmp[:], in0=x_f[:, i * C : (i + 1) * C],
                        scalar1=hpre_g[:, t8, i : i + 1], scalar2=None,
                        op0=ALU.mult,
                    )
                    nc.vector.tensor_tensor(
                        out=xi[:], in0=xi[:], in1=xtmp[:], op=ALU.add
                    )

                ptx = ps_tp.tile([P, 4 * P], F32, tag="tp")
                for cb in range(4):
                    nc.tensor.transpose(
                        ptx[:, cb * P : (cb + 1) * P],
                        xi[:, cb * P : (cb + 1) * P],
                        ident_sb[:],
                    )
                xiT = xinp.tile([P, HC, P], F32R, tag="xiT")
                nc.vector.tensor_copy(
                    out=xiT[:].rearrange("p a b -> p (a b)"), in_=ptx[:]
                )
                pf = ps_acc.tile([P, C], F32, tag="acc")
                for cb in range(4):
                    nc.tensor.matmul(
                        pf[:],
                        xiT[:, cb, :],
                        winner_sb[:, cb, :],
                        start=(cb == 0),
                        stop=(cb == 3),
                    )
                fout = fop.tile([P, C], F32R, tag="fout")
                nc.vector.tensor_copy(out=fout[:], in_=pf[:])

                # hyper-channel mixing on PE: x_out natural
                xo = xop.tile([P, HC, C], F32R, tag="xo")
                for o in range(HC):
                    po = ps_acc.tile([P, C], F32, tag="acc")
                    for i in range(HC):
                        nc.tensor.matmul(
                            po[:],
                            dg[:, o * HC + i, :],
                            x_r[:, i * C : (i + 1) * C],
                            start=(i == 0),
                            stop=False,
                        )
                    nc.tensor.matmul(
                        po[:], dg[:, 16 + o, :], fout[:], start=False, stop=True
                    )
                    nc.vector.tensor_copy(out=xo[:, o, :], in_=po[:])

                # transpose x_merge for the head matmul; copies on ACT
                xmt = xmtp.tile([P, KS, P], F32R, tag="xmt")
                for kb in range(KS // 4):
                    pt = ps_tp.tile([P, 4 * P], F32R, tag="tp")
                    for j in range(4):
                        ks = kb * 4 + j
                        o, cb = ks // 4, ks % 4
                        nc.tensor.transpose(
                            pt[:, j * P : (j + 1) * P],
                            xo[:, o, cb * P : (cb + 1) * P],
                            ident_r[:],
                        )
                    nc.scalar.copy(
                        out=xmt[:, 4 * kb : 4 * kb + 4, :].rearrange(
                            "p a b -> p (a b)"
                        ),
                        in_=pt[:],
                    )
                return xmt

            for g in range(n_groups):
                gssq4 = gsm.tile([P, group_tiles], F32, tag="gssq4")
                gmix = gsm.tile([P, group_tiles, M], F32, tag="gmix")
                for t8 in range(group_tiles):
                    stage1a(g * group_tiles + t8, t8, gssq4, gmix)
                sg_g, hpre_g, mat_g = batched_coeffs(group_tiles, gssq4, gmix)
                xmts = []
                for t8 in range(group_tiles):
                    xmts.append(
                        stage2(g * group_tiles + t8, t8, sg_g, hpre_g, mat_g)
                    )

                # ---- head matmul for this token group ----
                for vt in range(n_vt):
                    w_sb = wp.tile([P, KS, vt_size], F32R, tag="w")
                    for kq in range(4):
                        nc.sync.dma_start(
                            w_sb[:, 4 * kq : 4 * kq + 4, :],
                            wht_v[
                                :, 4 * kq : 4 * kq + 4,
                                vt * vt_size : (vt + 1) * vt_size,
                            ],
                        )
                    for t8 in range(group_tiles):
                        tt = g * group_tiles + t8
                        ph = ps_head.tile([P, vt_size], F32, tag="ph")
                        for ks in range(KS):
                            nc.tensor.matmul(
                                ph[:],
                                xmts[t8][:, ks, :],
                                w_sb[:, ks, :],
                                start=(ks == 0),
                                stop=(ks == KS - 1),
                            )
                        ost = ostp.tile([P, vt_size], F32, tag="ost")
                        nc.any.tensor_copy(out=ost[:], in_=ph[:])
                        nc.sync.dma_start(
                            out_d[
                                tt * P : (tt + 1) * P,
                                vt * vt_size : (vt + 1) * vt_size,
                            ],
                            ost[:],
                        )

    legalize_multiwait(nc)
    return nc


LAST_RESULT = None


def kernel(input_ids, embed, W_inner, W_head, phi, b,
           alpha_pre, alpha_post, alpha_res):
    global LAST_RESULT
    ids = np.asarray(input_ids).reshape(-1).astype(np.int32)
    B, S = np.asarray(input_ids).shape
    n_tok = ids.size
    n_tiles = n_tok // P
    embed = np.ascontiguousarray(np.asarray(embed, dtype=np.float32))
    vocab = embed.shape[0]
    vsh = vocab // N_CORES

    ids_pm = np.ascontiguousarray(ids.reshape(n_tiles, P).T)  # [128, n_tiles]
    wht_full = np.ascontiguousarray(np.asarray(W_head, np.float32).T)  # [K, vocab]
    winner = np.ascontiguousarray(np.asarray(W_inner, np.float32).T) * np.float32(2.0)
    phi_np = np.ascontiguousarray(np.asarray(phi, np.float32))
    params = np.zeros((P, 28), np.float32)
    params[:, :24] = np.asarray(b, np.float32)[None, :]
    params[:, 24] = np.float32(alpha_pre)
    params[:, 25] = np.float32(alpha_post)
    params[:, 26] = np.float32(alpha_res)
    params[:, 27] = np.float32(RMS_EPS)
    ident = np.eye(P, dtype=np.float32)

    nc = build_nc(n_tok=n_tok, vsh=vsh, embed_rows=vocab, group_tiles=8)

    in_maps = []
    for c in range(N_CORES):
        in_maps.append(
            {
                "ids": ids_pm,
                "embed": embed,
                "wht": np.ascontiguousarray(
                    wht_full[:, c * vsh : (c + 1) * vsh]
                ),
                "winner": winner,
                "phi": phi_np,
                "params": params,
                "ident": ident,
            }
        )
    res = run_bass_kernel_spmd(nc, in_maps, core_ids=list(range(N_CORES)))
    LAST_RESULT = res
    logits = np.concatenate(
        [res.results[c]["out"] for c in range(N_CORES)], axis=1
    )
    return logits.reshape(B, S, vocab).astype(np.float32)


# revision 11
# speedup vs baseline: 1.2348x; 1.2348x over previous
"""Trainium2 Bass kernel for MiniMHCLM.

Math (HC=4, C=512, K=HC*C=2048, VOCAB=32000, tokens N=B*S=4096):
  x = embed[ids]                               [N, K]
  invr = rsqrt(mean(x^2, -1) + eps)
  mix = (x @ phi) * invr                       [N, 24]
  h_pre  = sigmoid(mix[:, :4]*a_pre + b[:4]) + 0.01
  h_post = sigmoid(mix[:, 4:8]*a_post + b[4:8]) * 2
  h_res  = sinkhorn(mix[:, 8:24]*a_res + b[8:24], 8 iters)  [N,4,4]
  x_in  = sum_i h_pre[i] * x[:, i*C:(i+1)*C]
  f_out = x_in @ W_inner.T
  x_out[o] = sum_i h_res[o,i]*x[i] + h_post[o]*f_out
  logits = x_out.reshape(N, K) @ W_head.T      [N, VOCAB]

Distribution: all 8 cores run the identical coeff path over all tokens;
the head projection is column-sharded over vocab (4000 per core).

PE strategy: all heavy matmuls use the float32r dtype (fp32 stored,
PE rounds operands to 11 mantissa bits, runs at 1 cyc/row for moving
dim >= 256 - 4x faster than plain fp32, ~25x more accurate than bf16).
Per-token coefficient scaling is done ON the PE by using diag(h) as the
stationary operand: diag(h).T @ x = per-token-row scaled x, accumulated
in PSUM across hyper-channels. POST_MULT=2 is folded into W_inner.
"""

import sys

for _p in ("/opt/trn_rl_repo", "/root/.axon_site/_ro/trn_rl_repo"):
    if _p not in sys.path:
        sys.path.insert(0, _p)

import numpy as np

import concourse.bass as bass
import concourse.mybir as mybir
import concourse.tile as tile
from concourse.bass_utils import run_bass_kernel_spmd

F32 = mybir.dt.float32
F32R = mybir.dt.float32r
ALU = mybir.AluOpType
ACTF = mybir.ActivationFunctionType
AX = mybir.AxisListType

P = 128
HC, C = 4, 512
K = HC * C  # 2048
KS = K // P  # 16
M = HC * HC + 2 * HC  # 24
RMS_EPS = 1e-6
PRE_EPS = 0.01
TMAX = 8
N_CORES = 8


def legalize_multiwait(nc):
    """Split instructions carrying >1 semaphore wait.

    The walrus build in this image rejects instructions with more than
    one sem wait ("Too many sync wait commands"); Tile emits them
    freely. Move all but the last wait onto standalone InstEventSemaphore
    instructions inserted just before, on the same engine.
    """
    n_fixed = 0
    for fn in nc.m.functions:
        for blk in fn.blocks:
            new = []
            for ins in blk.instructions:
                si = ins.sync_info
                if si is not None and si.on_wait and len(si.on_wait) > 1:
                    waits = list(si.on_wait)
                    for j, w in enumerate(waits[:-1]):
                        es = mybir.InstEventSemaphore(
                            name=f"{ins.name}-w{j}",
                            ins=[],
                            outs=[],
                            sync_info=mybir.SyncInfo(on_wait=[w], on_update=[]),
                        )
                        es.engine = ins.engine
                        nc.register_instruction(es)
                        new.append(es)
                        n_fixed += 1
                    ins.sync_info = mybir.SyncInfo(
                        on_wait=[waits[-1]], on_update=list(si.on_update)
                    )
                new.append(ins)
            blk.instructions[:] = new
    return n_fixed


def build_nc(n_tok, vsh, embed_rows, group_tiles, vt_size=400):
    """Build the single-core Bass program (same on every core; only the
    wht input shard differs per core).

    The per-tile coefficient work is software-pipelined with a 1-tile
    skew: PE runs tile t's mixing/merge stage while DVE/ACT compute
    tile t+1's coefficients, keeping the PE dense (HAM stays warm).
    """
    n_tiles = n_tok // P
    assert n_tiles % group_tiles == 0
    n_groups = n_tiles // group_tiles
    assert vsh % vt_size == 0
    n_vt = vsh // vt_size

    nc = bass.Bass()

    ids_d = nc.dram_tensor("ids", [P, n_tiles], mybir.dt.int32, kind="ExternalInput")
    embed_d = nc.dram_tensor("embed", [embed_rows, K], F32R, kind="ExternalInput")
    wht_d = nc.dram_tensor("wht", [K, vsh], F32R, kind="ExternalInput")
    winner_d = nc.dram_tensor("winner", [C, C], F32R, kind="ExternalInput")
    phi_d = nc.dram_tensor("phi", [K, M], F32R, kind="ExternalInput")
    params_d = nc.dram_tensor("params", [P, 28], F32, kind="ExternalInput")
    ident_d = nc.dram_tensor("ident", [P, P], F32, kind="ExternalInput")
    out_d = nc.dram_tensor("out", [n_tok, vsh], F32, kind="ExternalOutput")

    wht_v = wht_d[:].rearrange("(ko p) v -> p ko v", p=P)  # [128, 16, vsh]

    with tile.TileContext(nc) as tc:
        with (
            tc.tile_pool(name="const", bufs=1) as cpool,
            tc.tile_pool(name="xg", bufs=3) as xg,
            tc.tile_pool(name="sm", bufs=2) as sm,
            tc.tile_pool(name="dg", bufs=2) as dgp,
            tc.tile_pool(name="xt", bufs=1) as xtp,
            tc.tile_pool(name="xin", bufs=2) as xinp,
            tc.tile_pool(name="fo", bufs=2) as fop,
            tc.tile_pool(name="xo", bufs=1) as xop,
            tc.tile_pool(name="xmt", bufs=group_tiles) as xmtp,
            tc.tile_pool(name="wp", bufs=2) as wp,
            tc.tile_pool(name="ost", bufs=2) as ostp,
            tc.tile_pool(name="ps_tp", bufs=2, space="PSUM") as ps_tp,
            tc.tile_pool(name="ps_acc", bufs=2, space="PSUM") as ps_acc,
            tc.tile_pool(name="ps_head", bufs=3, space="PSUM") as ps_head,
        ):
            # ---- constants ----
            phi_sb = cpool.tile([P, KS, M], F32R)
            nc.sync.dma_start(phi_sb[:], phi_d[:].rearrange("(ko p) m -> p ko m", p=P))
            winner_sb = cpool.tile([P, HC, C], F32R)
            nc.sync.dma_start(
                winner_sb[:], winner_d[:].rearrange("(ko p) c -> p ko c", p=P)
            )
            bvec = cpool.tile([P, 28], F32)
            nc.sync.dma_start(bvec[:], params_d[:])
            ids_sb = cpool.tile([P, n_tiles], mybir.dt.int32)
            nc.sync.dma_start(ids_sb[:], ids_d[:])
            ident_sb = cpool.tile([P, P], F32)
            nc.sync.dma_start(ident_sb[:], ident_d[:])
            ident_r = cpool.tile([P, P], F32R)
            nc.vector.tensor_copy(out=ident_r[:], in_=ident_sb[:])

            def stage1(tt):
                """Gather + RMS + x^T + mix + coefficients + diag + x_in.
                Returns per-tile state consumed by stage2."""
                # gather x rows (f32r tile; fp32 view for DVE/ACT)
                x_r = xg.tile([P, K], F32R, tag="x")
                nc.gpsimd.indirect_dma_start(
                    out=x_r[:],
                    out_offset=None,
                    in_=embed_d[:],
                    in_offset=bass.IndirectOffsetOnAxis(
                        ap=ids_sb[:, tt : tt + 1], axis=0
                    ),
                )
                x_f = x_r[:].bitcast(F32)

                # RMS (ACT square + accum, chunked: scratch is [P, C])
                ssq4 = sm.tile([P, HC], F32, tag="ssq4")
                for q in range(HC):
                    scratch = xinp.tile([P, C], F32, tag="xtmp")
                    nc.scalar.activation(
                        out=scratch[:],
                        in_=x_f[:, q * C : (q + 1) * C],
                        func=ACTF.Square,
                        accum_out=ssq4[:, q : q + 1],
                    )
                ssq = sm.tile([P, 1], F32, tag="ssq")
                nc.vector.tensor_reduce(
                    out=ssq[:], in_=ssq4[:], axis=AX.X, op=ALU.add
                )
                rms = sm.tile([P, 1], F32, tag="rms")
                nc.scalar.activation(
                    out=rms[:], in_=ssq[:], func=ACTF.Sqrt, scale=1.0 / K,
                    bias=bvec[:, 27:28],
                )
                invr = sm.tile([P, 1], F32, tag="invr")
                nc.vector.reciprocal(out=invr[:], in_=rms[:])

                # transpose x (for the mix matmul); copies on ACT
                xT = xtp.tile([P, KS, P], F32R, tag="xT")
                for kb in range(KS // 4):
                    pt = ps_tp.tile([P, 4 * P], F32R, tag="tp")
                    for j in range(4):
                        ks = kb * 4 + j
                        nc.tensor.transpose(
                            pt[:, j * P : (j + 1) * P],
                            x_r[:, ks * P : (ks + 1) * P],
                            ident_r[:],
                        )
                    nc.scalar.copy(
                        out=xT[:, 4 * kb : 4 * kb + 4, :].rearrange(
                            "p a b -> p (a b)"
                        ),
                        in_=pt[:],
                    )

                # mix = (x @ phi) * invr
                pm = ps_acc.tile([P, C], F32, tag="acc")
                for ks in range(KS):
                    nc.tensor.matmul(
                        pm[:, :M],
                        xT[:, ks, :],
                        phi_sb[:, ks, :],
                        start=(ks == 0),
                        stop=(ks == KS - 1),
                    )
                mix = sm.tile([P, M], F32, tag="mix")
                nc.vector.tensor_scalar(
                    out=mix[:], in0=pm[:, :M], scalar1=invr[:], scalar2=None,
                    op0=ALU.mult,
                )

                # coefficient logits
                lg = sm.tile([P, M], F32, tag="lg")
                nc.vector.tensor_scalar(
                    out=lg[:, 0:4], in0=mix[:, 0:4],
                    scalar1=bvec[:, 24:25], scalar2=None, op0=ALU.mult,
                )
                nc.vector.tensor_scalar(
                    out=lg[:, 4:8], in0=mix[:, 4:8],
                    scalar1=bvec[:, 25:26], scalar2=None, op0=ALU.mult,
                )
                nc.vector.tensor_scalar(
                    out=lg[:, 8:24], in0=mix[:, 8:24],
                    scalar1=bvec[:, 26:27], scalar2=None, op0=ALU.mult,
                )
                nc.vector.tensor_tensor(
                    out=lg[:], in0=lg[:], in1=bvec[:, 0:24], op=ALU.add
                )

                sg = sm.tile([P, 8], F32, tag="sg")
                nc.scalar.activation(out=sg[:], in_=lg[:, 0:8], func=ACTF.Sigmoid)
                hpre = sm.tile([P, 4], F32, tag="hpre")
                nc.vector.tensor_scalar(
                    out=hpre[:], in0=sg[:, 0:4], scalar1=PRE_EPS, scalar2=None,
                    op0=ALU.add,
                )

                # sinkhorn on mat = exp(res_logits)
                mat = sm.tile([P, HC, HC], F32, tag="mat")  # [p, o, i]
                nc.scalar.activation(
                    out=mat[:].rearrange("p a b -> p (a b)"), in_=lg[:, 8:24],
                    func=ACTF.Exp,
                )
                rsum = sm.tile([P, HC], F32, tag="rsum")
                rrec = sm.tile([P, HC], F32, tag="rrec")

                def row_norm():
                    nc.vector.tensor_reduce(
                        out=rsum[:], in_=mat[:], axis=AX.X, op=ALU.add
                    )
                    nc.vector.reciprocal(out=rrec[:], in_=rsum[:])
                    nc.vector.tensor_tensor(
                        out=mat[:], in0=mat[:],
                        in1=rrec[:, :, None].to_broadcast([P, HC, HC]),
                        op=ALU.mult,
                    )

                def col_norm():
                    nc.vector.tensor_reduce(
                        out=rsum[:], in_=mat[:].rearrange("p o i -> p i o"),
                        axis=AX.X, op=ALU.add,
                    )
                    nc.vector.reciprocal(out=rrec[:], in_=rsum[:])
                    nc.vector.tensor_tensor(
                        out=mat[:], in0=mat[:],
                        in1=rrec[:, None, :].to_broadcast([P, HC, HC]),
                        op=ALU.mult,
                    )

                row_norm()  # softmax denominator
                col_norm()
                for _ in range(TMAX - 1):
                    row_norm()
                    col_norm()

                # diag coefficient tiles: [h_res(16) | h_post(4)]
                dg = dgp.tile([P, 20, P], F32R, tag="dg")
                nc.vector.tensor_tensor(
                    out=dg[:, 0:16, :],
                    in0=ident_sb[:, None, :].to_broadcast([P, 16, P]),
                    in1=mat[:].rearrange("p o i -> p (o i)")[:, :, None]
                    .to_broadcast([P, 16, P]),
                    op=ALU.mult,
                )
                nc.vector.tensor_tensor(
                    out=dg[:, 16:20, :],
                    in0=ident_sb[:, None, :].to_broadcast([P, 4, P]),
                    in1=sg[:, 4:8, None].to_broadcast([P, 4, P]),
                    op=ALU.mult,
                )

                # x_in = sum_i h_pre[i] * x_i
                xi = xinp.tile([P, C], F32, tag="xi")
                nc.vector.tensor_scalar(
                    out=xi[:], in0=x_f[:, 0:C], scalar1=hpre[:, 0:1],
                    scalar2=None, op0=ALU.mult,
                )
                xtmp = xinp.tile([P, C], F32, tag="xtmp")
                for i in range(1, HC):
                    nc.vector.tensor_scalar(
                        out=xtmp[:], in0=x_f[:, i * C : (i + 1) * C],
                        scalar1=hpre[:, i : i + 1], scalar2=None, op0=ALU.mult,
                    )
                    nc.vector.tensor_tensor(
                        out=xi[:], in0=xi[:], in1=xtmp[:], op=ALU.add
                    )
                return {"x_r": x_r, "dg": dg, "xi": xi}

            def stage2(st):
                """x_in^T + f_out + PE mixing + merge transpose -> xmt."""
                x_r, dg, xi = st["x_r"], st["dg"], st["xi"]
                ptx = ps_tp.tile([P, 4 * P], F32, tag="tp")
                for cb in range(4):
                    nc.tensor.transpose(
                        ptx[:, cb * P : (cb + 1) * P],
                        xi[:, cb * P : (cb + 1) * P],
                        ident_sb[:],
                    )
                xiT = xinp.tile([P, HC, P], F32R, tag="xiT")
                nc.vector.tensor_copy(
                    out=xiT[:].rearrange("p a b -> p (a b)"), in_=ptx[:]
                )
                pf = ps_acc.tile([P, C], F32, tag="acc")
                for cb in range(4):
                    nc.tensor.matmul(
                        pf[:],
                        xiT[:, cb, :],
                        winner_sb[:, cb, :],
                        start=(cb == 0),
                        stop=(cb == 3),
                    )
                fout = fop.tile([P, C], F32R, tag="fout")
                nc.vector.tensor_copy(out=fout[:], in_=pf[:])

                # hyper-channel mixing on PE: x_out natural
                xo = xop.tile([P, HC, C], F32R, tag="xo")
                for o in range(HC):
                    po = ps_acc.tile([P, C], F32, tag="acc")
                    for i in range(HC):
                        nc.tensor.matmul(
                            po[:],
                            dg[:, o * HC + i, :],
                            x_r[:, i * C : (i + 1) * C],
                            start=(i == 0),
                            stop=False,
                        )
                    nc.tensor.matmul(
                        po[:], dg[:, 16 + o, :], fout[:], start=False, stop=True
                    )
                    nc.vector.tensor_copy(out=xo[:, o, :], in_=po[:])

                # transpose x_merge for the head matmul; copies on ACT
                xmt = xmtp.tile([P, KS, P], F32R, tag="xmt")
                for kb in range(KS // 4):
                    pt = ps_tp.tile([P, 4 * P], F32R, tag="tp")
                    for j in range(4):
                        ks = kb * 4 + j
                        o, cb = ks // 4, ks % 4
                        nc.tensor.transpose(
                            pt[:, j * P : (j + 1) * P],
                            xo[:, o, cb * P : (cb + 1) * P],
                            ident_r[:],
                        )
                    nc.scalar.copy(
                        out=xmt[:, 4 * kb : 4 * kb + 4, :].rearrange(
                            "p a b -> p (a b)"
                        ),
                        in_=pt[:],
                    )
                return xmt

            for g in range(n_groups):
                xmts = []
                pend = None
                for t8 in range(group_tiles):
                    tt = g * group_tiles + t8
                    st = stage1(tt)
                    if pend is not None:
                        xmts.append(stage2(pend))
                    pend = st
                xmts.append(stage2(pend))

                # ---- head matmul for this token group ----
                for vt in range(n_vt):
                    w_sb = wp.tile([P, KS, vt_size], F32R, tag="w")
                    for kq in range(4):
                        nc.sync.dma_start(
                            w_sb[:, 4 * kq : 4 * kq + 4, :],
                            wht_v[
                                :, 4 * kq : 4 * kq + 4,
                                vt * vt_size : (vt + 1) * vt_size,
                            ],
                        )
                    for t8 in range(group_tiles):
                        tt = g * group_tiles + t8
                        ph = ps_head.tile([P, vt_size], F32, tag="ph")
                        for ks in range(KS):
                            nc.tensor.matmul(
                                ph[:],
                                xmts[t8][:, ks, :],
                                w_sb[:, ks, :],
                                start=(ks == 0),
                                stop=(ks == KS - 1),
                            )
                        ost = ostp.tile([P, vt_size], F32, tag="ost")
                        nc.any.tensor_copy(out=ost[:], in_=ph[:])
                        nc.sync.dma_start(
                            out_d[
                                tt * P : (tt + 1) * P,
                                vt * vt_size : (vt + 1) * vt_size,
                            ],
                            ost[:],
                        )

    legalize_multiwait(nc)
    return nc


LAST_RESULT = None


def kernel(input_ids, embed, W_inner, W_head, phi, b,
           alpha_pre, alpha_post, alpha_res):
    global LAST_RESULT
    ids = np.asarray(input_ids).reshape(-1).astype(np.int32)
    B, S = np.asarray(input_ids).shape
    n_tok = ids.size
    n_tiles = n_tok // P
    embed = np.ascontiguousarray(np.asarray(embed, dtype=np.float32))
    vocab = embed.shape[0]
    vsh = vocab // N_CORES

    ids_pm = np.ascontiguousarray(ids.reshape(n_tiles, P).T)  # [128, n_tiles]
    wht_full = np.ascontiguousarray(np.asarray(W_head, np.float32).T)  # [K, vocab]
    winner = np.ascontiguousarray(np.asarray(W_inner, np.float32).T) * np.float32(2.0)
    phi_np = np.ascontiguousarray(np.asarray(phi, np.float32))
    params = np.zeros((P, 28), np.float32)
    params[:, :24] = np.asarray(b, np.float32)[None, :]
    params[:, 24] = np.float32(alpha_pre)
    params[:, 25] = np.float32(alpha_post)
    params[:, 26] = np.float32(alpha_res)
    params[:, 27] = np.float32(RMS_EPS)
    ident = np.eye(P, dtype=np.float32)

    nc = build_nc(n_tok=n_tok, vsh=vsh, embed_rows=vocab, group_tiles=8)

    in_maps = []
    for c in range(N_CORES):
        in_maps.append(
            {
                "ids": ids_pm,
                "embed": embed,
                "wht": np.ascontiguousarray(
                    wht_full[:, c * vsh : (c + 1) * vsh]
                ),
                "winner": winner,
                "phi": phi_np,
                "params": params,
                "ident": ident,
            }
        )
    res = run_bass_kernel_spmd(nc, in_maps, core_ids=list(range(N_CORES)))
    LAST_RESULT = res
    logits = np.concatenate(
        [res.results[c]["out"] for c in range(N_CORES)], axis=1
    )
    return logits.reshape(B, S, vocab).astype(np.float32)
